# revision 16
# baseline (speedup 1.0000x reference)
"""Low-rank RNN (h' = 0.9h + 0.1*tanh(h) @ J^T + 0.1*u, J = m n^T rank-8)
on 8 Trainium2 NeuronCores, data-parallel over batch.

v6 layout per core (batch shard Bs=8, T=512, H=1024=8x128, D=128, R=8):
  state h lives in PSUM: vp[p, 8c+b] = h[b, 128c+p]
  per step chain:
    tanh (ACT, PSUM->SBUF, bf16)
    -> mm1: 8 bf16 matmuls in 4 rounds x 2 column-halves of the PE array;
       half g accumulates s_g = sum_k n_{2k+g}^T th_{2k+g} (j-replicated)
       into srep2[64g+8j+r, :]
    -> mask (DVE, PSUM->SBUF bf16): rhs2[64g+8j+r, 8c+b] = s_g[b,r]*[j==c]
    -> mm2: vp' += s2x^T @ rhs2 (bf16); s2x[64g+8j+r, p] = 0.1*m[128j+p, r]
  The linear part of the update rides PSUM accumulation as bf16 identity
  matmuls with NO per-step vector math:
    u'_t injected straight from the staged bf16 u tensor (exact);
    0.9*h injected from bf16 hi/lo ring copies of h (DVE: hi = bf16(h),
    lo = bf16(h - hi)) through two diagonal weights a = bf16(0.9),
    b = bf16(0.9 - a)  (a(hi+lo) + b*hi = 0.9h to ~2^-17).
  The hi/lo ring doubles as output staging: contiguous 4KB-per-partition
  DMAs of bf16 pairs; the host adds hi+lo and un-permutes.
  The u' staging matmuls are interleaved into the recurrence PE idle time,
  their PSUM->SBUF copies ride the ACT idle time.
"""

import numpy as np

B, T, D, H, R = 64, 512, 128, 1024, 8
NC = 8            # cores
BS = B // NC      # batch per core = 8
C = H // 128      # h chunks = 8
ALPHA = 0.1
DECAY = 1.0 - ALPHA

_CACHE = {}


def build(T_steps=T, ring=16, debug=False):
    import concourse.mybir as mybir
    import concourse.tile as tile
    from concourse import bacc

    f32 = mybir.dt.float32
    bf16 = mybir.dt.bfloat16
    AF = mybir.ActivationFunctionType
    OP = mybir.AluOpType

    nc = bacc.Bacc("TRN2", target_bir_lowering=False, debug=debug)

    TB = T_steps * BS                       # columns of xt / u
    BLK = min(512, TB)
    NBLK = TB // BLK
    STEPS_PER_BLK = BLK // BS

    xt_d = nc.dram_tensor("xt", [D, TB], bf16, kind="ExternalInput")
    itp_d = nc.dram_tensor("itp", [D, H], bf16, kind="ExternalInput")
    nst_d = nc.dram_tensor("nst", [128, 4 * 2 * 64], bf16, kind="ExternalInput")
    s2x_d = nc.dram_tensor("s2x", [128, 128], bf16, kind="ExternalInput")
    mask_d = nc.dram_tensor("maskx", [128, C * BS], bf16, kind="ExternalInput")
    id_d = nc.dram_tensor("id128", [128, 128], bf16, kind="ExternalInput")
    id32_d = nc.dram_tensor("id32", [128, 128], f32, kind="ExternalInput")
    out_d = nc.dram_tensor("out", [128, T_steps * C * BS], f32, kind="ExternalOutput")

    CB = C * BS  # 64

    with tile.TileContext(nc) as tc:
        with (
            tc.tile_pool(name="const", bufs=1) as constp,
            tc.tile_pool(name="upool", bufs=1) as upool,
            tc.tile_pool(name="xpool", bufs=1) as xpool,
            tc.tile_pool(name="th", bufs=3) as thp,
            tc.tile_pool(name="rhs2", bufs=3) as rhs2p,
            tc.tile_pool(name="hxp", bufs=3) as hxp,
            tc.tile_pool(name="ring", bufs=2) as ringp,
            tc.tile_pool(name="ps_s", bufs=2, space="PSUM") as ps_s,
            tc.tile_pool(name="ps_v", bufs=3, space="PSUM") as ps_v,
            tc.tile_pool(name="ps_u", bufs=2, space="PSUM") as ps_u,
            tc.tile_pool(name="ps_w", bufs=1, space="PSUM") as ps_w,
        ):
            # ---- constants ----
            nst_sb = constp.tile([128, 4, 2, 64], bf16)
            s2x_sb = constp.tile([128, 128], bf16)
            mask_sb = constp.tile([128, CB], bf16)
            id_sb = constp.tile([128, 128], bf16)
            id32_sb = constp.tile([128, 128], f32)
            id_r = constp.tile([128, 128], mybir.dt.float32r)
            itp_sb = constp.tile([D, H], bf16)
            xt_sb = xpool.tile([D, NBLK, BLK], bf16)
            nc.sync.dma_start(itp_sb[:], itp_d[:])
            nc.sync.dma_start(xt_sb[:, 0, :], xt_d[:, 0:BLK])
            nc.sync.dma_start(nst_sb[:].rearrange("p k g f -> p (k g f)"), nst_d[:])
            nc.sync.dma_start(s2x_sb[:], s2x_d[:])
            nc.sync.dma_start(mask_sb[:], mask_d[:])
            nc.sync.dma_start(id_sb[:], id_d[:])
            nc.sync.dma_start(id32_sb[:], id32_d[:])
            for blk in range(1, NBLK):
                nc.sync.dma_start(
                    xt_sb[:, blk, :], xt_d[:, blk * BLK:(blk + 1) * BLK]
                )

            # u'[p, c, t*BS+b] = 0.1 * u[b, t, 128c+p], staged per (blk, c)
            u_sb = upool.tile([128, C, TB], bf16)

            def u_matmul(blk, c):
                up = ps_u.tile([128, BLK], f32)
                nc.tensor.matmul(
                    up[:],
                    itp_sb[:, c * 128:(c + 1) * 128],
                    xt_sb[:, blk, :],
                    start=True, stop=True,
                )
                return up

            def u_copy(blk, c, up):
                dst = u_sb[:, c, blk * BLK:(blk + 1) * BLK]
                nc.vector.tensor_copy(dst, up[:])

            # one-time: f32r-rounded identity for the hx injection
            nc.vector.tensor_copy(id_r[:], id32_sb[:])

            # HAM warmup: ~3.5us of back-to-back matmuls on the identity so
            # the PE clock-gate opens before the u staging and recurrence
            warm = ps_w.tile([128, CB], f32)
            for _ in range(52):
                nc.tensor.matmul(warm[:], id_sb[:], id_sb[:, 0:CB],
                                 start=True, stop=True)

            # prologue: block 0 only (covers steps 0..STEPS_PER_BLK-1)
            for c in range(C):
                up = u_matmul(0, c)
                u_copy(0, c, up)

            # interleave schedule for blocks 1..NBLK-1:
            #   chunk c of block k rides step 64*(k-1) + 8 + 4*c
            u_sched = {}
            for k in range(1, NBLK):
                for c in range(C):
                    u_sched[(k - 1) * STEPS_PER_BLK + 8 + 4 * c] = (k, c)

            u_v = u_sb[:].rearrange("p c (t b) -> p c t b", b=BS)

            # ---- recurrence ----
            ring_tiles = []

            def ring_slot(t):
                return divmod(t, ring)

            def stage_out(t, vp_t):
                """ACT-copy h_t from PSUM into the ring; DMA full groups."""
                g, s = ring_slot(t)
                nc.scalar.activation(ring_tiles[g][:, s, :], vp_t[:], AF.Copy)
                if s == ring - 1 or t == T_steps - 1:
                    n_t = s + 1
                    nc.sync.dma_start(
                        out_d[:, g * ring * CB:(g * ring + n_t) * CB],
                        ring_tiles[g][:, :n_t, :].rearrange("p t f -> p (t f)"),
                    )

            vp_prev = None
            for t in range(T_steps):
                g, s = ring_slot(t)
                if s == 0:
                    rt = ringp.tile([128, ring, CB], f32, tag="ring")
                    ring_tiles.append(rt)

                vp = ps_v.tile([128, CB], f32)
                if t == 0:
                    # h_0 = u'_0  (reference: h_1 = 0.1*u_0 with zero init)
                    nc.tensor.matmul(vp[:], id_sb[:], u_v[:, :, 0, :],
                                     start=True, stop=True)
                else:
                    # chain 1: th = tanh(h) directly off PSUM, cast to bf16
                    th = thp.tile([128, C, BS], bf16)
                    nc.scalar.activation(
                        th[:].rearrange("p c b -> p (c b)"), vp_prev[:], AF.Tanh
                    )
                    # off-chain: hx = 0.9*h + u'_t (DVE, f32r out, after tanh)
                    hx = hxp.tile([128, CB], mybir.dt.float32r)
                    nc.vector.scalar_tensor_tensor(
                        hx[:].rearrange("p (c b) -> p c b", b=BS),
                        vp_prev[:].rearrange("p (c b) -> p c b", b=BS),
                        DECAY, u_v[:, :, t, :], OP.mult, OP.add,
                    )
                    # off-chain: stage h_{t-1} to the output ring (ACT)
                    stage_out(t - 1, vp_prev)
                    # chain 2: srep2[64g+8j+r, :] = s_g replicated (8 bf16 MMs)
                    srep2 = ps_s.tile([128, CB], f32)
                    for k in range(4):
                        for gg in range(2):
                            nc.tensor.matmul(
                                srep2[64 * gg:64 * (gg + 1), :],
                                nst_sb[:, k, gg, :],
                                th[:, 2 * k + gg:2 * k + gg + 1, :]
                                .to_broadcast((128, C, BS)),
                                start=(k == 0), stop=(k == 3),
                            )
                    # inject hx = 0.9h + u' (single f32r identity matmul)
                    nc.tensor.matmul(vp[:], id_r[:], hx[:], start=True, stop=False)
                    # chain 3: blockdiag mask, psum -> sbuf, bf16
                    rhs2 = rhs2p.tile([128, CB], bf16)
                    nc.vector.tensor_tensor(rhs2[:], srep2[:], mask_sb[:], OP.mult)
                    # chain 4: vp += s2x^T @ rhs2
                    nc.tensor.matmul(vp[:], s2x_sb[:], rhs2[:], start=False, stop=True)

                usched = u_sched.get(t)
                if usched is not None:
                    up_pend = u_matmul(*usched)   # rides the step-boundary idle
                    u_copy(*usched, up_pend)

                vp_prev = vp

            stage_out(T_steps - 1, vp_prev)

    nc.compile()
    return nc


def prep_inputs(x, m, n, I, T_steps=T):
    """Host-side shard + layout prep (pure data marshaling)."""
    import ml_dtypes
    bf16 = ml_dtypes.bfloat16

    x = np.asarray(x, np.float32)
    m = np.asarray(m, np.float32)
    n = np.asarray(n, np.float32)
    I = np.asarray(I, np.float32)

    itp = np.ascontiguousarray((ALPHA * I).T).astype(bf16)      # [D, H]
    # nst[p, k, g, 8j+r] = n[256k+128g+p, r]  (replicated over j)
    nst = np.empty((128, 4, 2, 64), np.float32)
    for k in range(4):
        for g in range(2):
            blk = n[256 * k + 128 * g: 256 * k + 128 * g + 128, :]  # [128, 8]
            nst[:, k, g, :] = np.tile(blk, (1, 8))
    # s2x[64g+8j+r, p] = 0.1*m[128j+p, r]
    s2 = (ALPHA * m).reshape(C, 128, R).transpose(0, 2, 1).reshape(C * R, 128)
    s2x = np.tile(s2, (2, 1))                                   # [128, 128]
    maskx = np.tile(
        np.kron(np.eye(C, dtype=np.float32), np.ones((R, BS), np.float32)), (2, 1)
    )                                                           # [128, 64]

    nst_b = nst.reshape(128, 4 * 2 * 64).astype(bf16)
    s2x_b = s2x.astype(bf16)
    maskx_b = maskx.astype(bf16)
    eye = np.eye(128, dtype=np.float32)
    id_b = eye.astype(bf16)

    in_maps = []
    for core in range(NC):
        xs = x[core * BS:(core + 1) * BS, :T_steps]             # [BS, Ts, D]
        xt = np.ascontiguousarray(
            xs.transpose(2, 1, 0).reshape(D, T_steps * BS)
        ).astype(bf16)
        in_maps.append({
            "xt": xt, "itp": itp, "nst": nst_b, "s2x": s2x_b,
            "maskx": maskx_b, "id128": id_b, "id32": eye,
        })
    return in_maps


def unshard_out(res_core, T_steps=T):
    """[128, T*64] device layout -> [BS, T, H] full layout for one core."""
    a = res_core.reshape(128, T_steps, C, BS)        # [p, t, c, b]
    return np.ascontiguousarray(a.transpose(3, 1, 2, 0)).reshape(BS, T_steps, H)


def kernel(x, m, n, I):
    from concourse.bass_utils import run_bass_kernel_spmd

    if "nc" not in _CACHE:
        _CACHE["nc"] = build()
    nc = _CACHE["nc"]

    in_maps = prep_inputs(x, m, n, I)
    res = run_bass_kernel_spmd(nc, in_maps, core_ids=list(range(NC)))
    out = np.concatenate(
        [unshard_out(res.results[c]["out"]) for c in range(NC)], axis=0
    )
    return out


# revision 17
# speedup vs baseline: 1.0016x; 1.0016x over previous
"""Low-rank RNN (h' = 0.9h + 0.1*tanh(h) @ J^T + 0.1*u, J = m n^T rank-8)
on 8 Trainium2 NeuronCores, data-parallel over batch.

v6 layout per core (batch shard Bs=8, T=512, H=1024=8x128, D=128, R=8):
  state h lives in PSUM: vp[p, 8c+b] = h[b, 128c+p]
  per step chain:
    tanh (ACT, PSUM->SBUF, bf16)
    -> mm1: 8 bf16 matmuls in 4 rounds x 2 column-halves of the PE array;
       half g accumulates s_g = sum_k n_{2k+g}^T th_{2k+g} (j-replicated)
       into srep2[64g+8j+r, :]
    -> mask (DVE, PSUM->SBUF bf16): rhs2[64g+8j+r, 8c+b] = s_g[b,r]*[j==c]
    -> mm2: vp' += s2x^T @ rhs2 (bf16); s2x[64g+8j+r, p] = 0.1*m[128j+p, r]
  The linear part of the update rides PSUM accumulation as bf16 identity
  matmuls with NO per-step vector math:
    u'_t injected straight from the staged bf16 u tensor (exact);
    0.9*h injected from bf16 hi/lo ring copies of h (DVE: hi = bf16(h),
    lo = bf16(h - hi)) through two diagonal weights a = bf16(0.9),
    b = bf16(0.9 - a)  (a(hi+lo) + b*hi = 0.9h to ~2^-17).
  The hi/lo ring doubles as output staging: contiguous 4KB-per-partition
  DMAs of bf16 pairs; the host adds hi+lo and un-permutes.
  The u' staging matmuls are interleaved into the recurrence PE idle time,
  their PSUM->SBUF copies ride the ACT idle time.
"""

import numpy as np

B, T, D, H, R = 64, 512, 128, 1024, 8
NC = 8            # cores
BS = B // NC      # batch per core = 8
C = H // 128      # h chunks = 8
ALPHA = 0.1
DECAY = 1.0 - ALPHA

_CACHE = {}


def build(T_steps=T, ring=16, debug=False):
    import concourse.mybir as mybir
    import concourse.tile as tile
    from concourse import bacc

    f32 = mybir.dt.float32
    bf16 = mybir.dt.bfloat16
    AF = mybir.ActivationFunctionType
    OP = mybir.AluOpType

    nc = bacc.Bacc("TRN2", target_bir_lowering=False, debug=debug)

    TB = T_steps * BS                       # columns of xt / u
    BLK = min(512, TB)
    NBLK = TB // BLK
    STEPS_PER_BLK = BLK // BS

    xt_d = nc.dram_tensor("xt", [D, TB], bf16, kind="ExternalInput")
    itp_d = nc.dram_tensor("itp", [D, H], bf16, kind="ExternalInput")
    nst_d = nc.dram_tensor("nst", [128, 4 * 2 * 64], bf16, kind="ExternalInput")
    s2x_d = nc.dram_tensor("s2x", [128, 128], bf16, kind="ExternalInput")
    mask_d = nc.dram_tensor("maskx", [128, C * BS], bf16, kind="ExternalInput")
    id_d = nc.dram_tensor("id128", [128, 128], bf16, kind="ExternalInput")
    id32_d = nc.dram_tensor("id32", [128, 128], f32, kind="ExternalInput")
    out_d = nc.dram_tensor("out", [128, T_steps * C * BS], f32, kind="ExternalOutput")

    CB = C * BS  # 64

    with tile.TileContext(nc) as tc:
        with (
            tc.tile_pool(name="const", bufs=1) as constp,
            tc.tile_pool(name="upool", bufs=1) as upool,
            tc.tile_pool(name="xpool", bufs=1) as xpool,
            tc.tile_pool(name="th", bufs=3) as thp,
            tc.tile_pool(name="rhs2", bufs=3) as rhs2p,
            tc.tile_pool(name="hxp", bufs=3) as hxp,
            tc.tile_pool(name="ring", bufs=2) as ringp,
            tc.tile_pool(name="ps_s", bufs=2, space="PSUM") as ps_s,
            tc.tile_pool(name="ps_v", bufs=3, space="PSUM") as ps_v,
            tc.tile_pool(name="ps_u", bufs=2, space="PSUM") as ps_u,
        ):
            # ---- constants ----
            nst_sb = constp.tile([128, 4, 2, 64], bf16)
            s2x_sb = constp.tile([128, 128], bf16)
            mask_sb = constp.tile([128, CB], bf16)
            id_sb = constp.tile([128, 128], bf16)
            id32_sb = constp.tile([128, 128], f32)
            id_r = constp.tile([128, 128], mybir.dt.float32r)
            itp_sb = constp.tile([D, H], bf16)
            xt_sb = xpool.tile([D, NBLK, BLK], bf16)
            nc.sync.dma_start(itp_sb[:], itp_d[:])
            nc.sync.dma_start(xt_sb[:, 0, :], xt_d[:, 0:BLK])
            nc.sync.dma_start(nst_sb[:].rearrange("p k g f -> p (k g f)"), nst_d[:])
            nc.sync.dma_start(s2x_sb[:], s2x_d[:])
            nc.sync.dma_start(mask_sb[:], mask_d[:])
            nc.sync.dma_start(id_sb[:], id_d[:])
            nc.sync.dma_start(id32_sb[:], id32_d[:])
            for blk in range(1, NBLK):
                nc.sync.dma_start(
                    xt_sb[:, blk, :], xt_d[:, blk * BLK:(blk + 1) * BLK]
                )

            # u'[p, c, t*BS+b] = 0.1 * u[b, t, 128c+p], staged per (blk, c)
            u_sb = upool.tile([128, C, TB], bf16)

            def u_matmul(blk, c):
                up = ps_u.tile([128, BLK], f32)
                nc.tensor.matmul(
                    up[:],
                    itp_sb[:, c * 128:(c + 1) * 128],
                    xt_sb[:, blk, :],
                    start=True, stop=True,
                )
                return up

            def u_copy(blk, c, up):
                dst = u_sb[:, c, blk * BLK:(blk + 1) * BLK]
                nc.vector.tensor_copy(dst, up[:])

            # one-time: f32r-rounded identity for the hx injection
            nc.vector.tensor_copy(id_r[:], id32_sb[:])

            # prologue: block 0 only (covers steps 0..STEPS_PER_BLK-1)
            for c in range(C):
                up = u_matmul(0, c)
                u_copy(0, c, up)

            # interleave schedule for blocks 1..NBLK-1:
            #   chunk c of block k rides step 64*(k-1) + 8 + 4*c
            u_sched = {}
            for k in range(1, NBLK):
                for c in range(C):
                    u_sched[(k - 1) * STEPS_PER_BLK + 8 + 4 * c] = (k, c)

            u_v = u_sb[:].rearrange("p c (t b) -> p c t b", b=BS)

            # ---- recurrence ----
            ring_tiles = []

            def ring_slot(t):
                return divmod(t, ring)

            def stage_out(t, vp_t):
                """ACT-copy h_t from PSUM into the ring; DMA full groups."""
                g, s = ring_slot(t)
                nc.scalar.activation(ring_tiles[g][:, s, :], vp_t[:], AF.Copy)
                if s == ring - 1 or t == T_steps - 1:
                    n_t = s + 1
                    nc.sync.dma_start(
                        out_d[:, g * ring * CB:(g * ring + n_t) * CB],
                        ring_tiles[g][:, :n_t, :].rearrange("p t f -> p (t f)"),
                    )

            vp_prev = None
            for t in range(T_steps):
                g, s = ring_slot(t)
                if s == 0:
                    rt = ringp.tile([128, ring, CB], f32, tag="ring")
                    ring_tiles.append(rt)

                vp = ps_v.tile([128, CB], f32)
                if t == 0:
                    # h_0 = u'_0  (reference: h_1 = 0.1*u_0 with zero init)
                    nc.tensor.matmul(vp[:], id_sb[:], u_v[:, :, 0, :],
                                     start=True, stop=True)
                else:
                    # chain 1: th = tanh(h) directly off PSUM, cast to bf16
                    th = thp.tile([128, C, BS], bf16)
                    nc.scalar.activation(
                        th[:].rearrange("p c b -> p (c b)"), vp_prev[:], AF.Tanh
                    )
                    # off-chain: hx = 0.9*h + u'_t (DVE, f32r out, after tanh)
                    hx = hxp.tile([128, CB], mybir.dt.float32r)
                    nc.vector.scalar_tensor_tensor(
                        hx[:].rearrange("p (c b) -> p c b", b=BS),
                        vp_prev[:].rearrange("p (c b) -> p c b", b=BS),
                        DECAY, u_v[:, :, t, :], OP.mult, OP.add,
                    )
                    # off-chain: stage h_{t-1} to the output ring (ACT)
                    stage_out(t - 1, vp_prev)
                    # chain 2: srep2[64g+8j+r, :] = s_g replicated (8 bf16 MMs)
                    srep2 = ps_s.tile([128, CB], f32)
                    for k in range(4):
                        for gg in range(2):
                            nc.tensor.matmul(
                                srep2[64 * gg:64 * (gg + 1), :],
                                nst_sb[:, k, gg, :],
                                th[:, 2 * k + gg:2 * k + gg + 1, :]
                                .to_broadcast((128, C, BS)),
                                start=(k == 0), stop=(k == 3),
                            )
                    # inject hx = 0.9h + u' (single f32r identity matmul)
                    nc.tensor.matmul(vp[:], id_r[:], hx[:], start=True, stop=False)
                    # chain 3: blockdiag mask, psum -> sbuf, bf16
                    rhs2 = rhs2p.tile([128, CB], bf16)
                    nc.vector.tensor_tensor(rhs2[:], srep2[:], mask_sb[:], OP.mult)
                    # chain 4: vp += s2x^T @ rhs2
                    nc.tensor.matmul(vp[:], s2x_sb[:], rhs2[:], start=False, stop=True)

                usched = u_sched.get(t)
                if usched is not None:
                    with tc.high_priority(offset=-1000000):
                        up_pend = u_matmul(*usched)
                        u_copy(*usched, up_pend)

                vp_prev = vp

            stage_out(T_steps - 1, vp_prev)

    nc.compile()
    return nc


def prep_inputs(x, m, n, I, T_steps=T):
    """Host-side shard + layout prep (pure data marshaling)."""
    import ml_dtypes
    bf16 = ml_dtypes.bfloat16

    x = np.asarray(x, np.float32)
    m = np.asarray(m, np.float32)
    n = np.asarray(n, np.float32)
    I = np.asarray(I, np.float32)

    itp = np.ascontiguousarray((ALPHA * I).T).astype(bf16)      # [D, H]
    # nst[p, k, g, 8j+r] = n[256k+128g+p, r]  (replicated over j)
    nst = np.empty((128, 4, 2, 64), np.float32)
    for k in range(4):
        for g in range(2):
            blk = n[256 * k + 128 * g: 256 * k + 128 * g + 128, :]  # [128, 8]
            nst[:, k, g, :] = np.tile(blk, (1, 8))
    # s2x[64g+8j+r, p] = 0.1*m[128j+p, r]
    s2 = (ALPHA * m).reshape(C, 128, R).transpose(0, 2, 1).reshape(C * R, 128)
    s2x = np.tile(s2, (2, 1))                                   # [128, 128]
    maskx = np.tile(
        np.kron(np.eye(C, dtype=np.float32), np.ones((R, BS), np.float32)), (2, 1)
    )                                                           # [128, 64]

    nst_b = nst.reshape(128, 4 * 2 * 64).astype(bf16)
    s2x_b = s2x.astype(bf16)
    maskx_b = maskx.astype(bf16)
    eye = np.eye(128, dtype=np.float32)
    id_b = eye.astype(bf16)

    in_maps = []
    for core in range(NC):
        xs = x[core * BS:(core + 1) * BS, :T_steps]             # [BS, Ts, D]
        xt = np.ascontiguousarray(
            xs.transpose(2, 1, 0).reshape(D, T_steps * BS)
        ).astype(bf16)
        in_maps.append({
            "xt": xt, "itp": itp, "nst": nst_b, "s2x": s2x_b,
            "maskx": maskx_b, "id128": id_b, "id32": eye,
        })
    return in_maps


def unshard_out(res_core, T_steps=T):
    """[128, T*64] device layout -> [BS, T, H] full layout for one core."""
    a = res_core.reshape(128, T_steps, C, BS)        # [p, t, c, b]
    return np.ascontiguousarray(a.transpose(3, 1, 2, 0)).reshape(BS, T_steps, H)


def kernel(x, m, n, I):
    from concourse.bass_utils import run_bass_kernel_spmd

    if "nc" not in _CACHE:
        _CACHE["nc"] = build()
    nc = _CACHE["nc"]

    in_maps = prep_inputs(x, m, n, I)
    res = run_bass_kernel_spmd(nc, in_maps, core_ids=list(range(NC)))
    out = np.concatenate(
        [unshard_out(res.results[c]["out"]) for c in range(NC)], axis=0
    )
    return out


# revision 18
# speedup vs baseline: 1.0037x; 1.0021x over previous
"""Low-rank RNN (h' = 0.9h + 0.1*tanh(h) @ J^T + 0.1*u, J = m n^T rank-8)
on 8 Trainium2 NeuronCores, data-parallel over batch.

v6 layout per core (batch shard Bs=8, T=512, H=1024=8x128, D=128, R=8):
  state h lives in PSUM: vp[p, 8c+b] = h[b, 128c+p]
  per step chain:
    tanh (ACT, PSUM->SBUF, bf16)
    -> mm1: 8 bf16 matmuls in 4 rounds x 2 column-halves of the PE array;
       half g accumulates s_g = sum_k n_{2k+g}^T th_{2k+g} (j-replicated)
       into srep2[64g+8j+r, :]
    -> mask (DVE, PSUM->SBUF bf16): rhs2[64g+8j+r, 8c+b] = s_g[b,r]*[j==c]
    -> mm2: vp' += s2x^T @ rhs2 (bf16); s2x[64g+8j+r, p] = 0.1*m[128j+p, r]
  The linear part of the update rides PSUM accumulation as bf16 identity
  matmuls with NO per-step vector math:
    u'_t injected straight from the staged bf16 u tensor (exact);
    0.9*h injected from bf16 hi/lo ring copies of h (DVE: hi = bf16(h),
    lo = bf16(h - hi)) through two diagonal weights a = bf16(0.9),
    b = bf16(0.9 - a)  (a(hi+lo) + b*hi = 0.9h to ~2^-17).
  The hi/lo ring doubles as output staging: contiguous 4KB-per-partition
  DMAs of bf16 pairs; the host adds hi+lo and un-permutes.
  The u' staging matmuls are interleaved into the recurrence PE idle time,
  their PSUM->SBUF copies ride the ACT idle time.
"""

import numpy as np

B, T, D, H, R = 64, 512, 128, 1024, 8
NC = 8            # cores
BS = B // NC      # batch per core = 8
C = H // 128      # h chunks = 8
ALPHA = 0.1
DECAY = 1.0 - ALPHA

_CACHE = {}


def build(T_steps=T, ring=16, debug=False):
    import concourse.mybir as mybir
    import concourse.tile as tile
    from concourse import bacc

    f32 = mybir.dt.float32
    bf16 = mybir.dt.bfloat16
    AF = mybir.ActivationFunctionType
    OP = mybir.AluOpType

    nc = bacc.Bacc("TRN2", target_bir_lowering=False, debug=debug)

    TB = T_steps * BS                       # columns of xt / u
    BLK = min(512, TB)
    NBLK = TB // BLK
    STEPS_PER_BLK = BLK // BS

    xt_d = nc.dram_tensor("xt", [D, TB], bf16, kind="ExternalInput")
    itp_d = nc.dram_tensor("itp", [D, H], bf16, kind="ExternalInput")
    nst_d = nc.dram_tensor("nst", [128, 4 * 2 * 64], bf16, kind="ExternalInput")
    s2x_d = nc.dram_tensor("s2x", [128, 128], bf16, kind="ExternalInput")
    mask_d = nc.dram_tensor("maskx", [128, C * BS], bf16, kind="ExternalInput")
    id_d = nc.dram_tensor("id128", [128, 128], bf16, kind="ExternalInput")
    id32_d = nc.dram_tensor("id32", [128, 128], f32, kind="ExternalInput")
    out_d = nc.dram_tensor("out", [128, T_steps * C * BS], f32, kind="ExternalOutput")

    CB = C * BS  # 64

    with tile.TileContext(nc) as tc:
        with (
            tc.tile_pool(name="const", bufs=1) as constp,
            tc.tile_pool(name="upool", bufs=1) as upool,
            tc.tile_pool(name="xpool", bufs=1) as xpool,
            tc.tile_pool(name="th", bufs=3) as thp,
            tc.tile_pool(name="rhs2", bufs=3) as rhs2p,
            tc.tile_pool(name="hxp", bufs=3) as hxp,
            tc.tile_pool(name="ring", bufs=2) as ringp,
            tc.tile_pool(name="ps_s", bufs=2, space="PSUM") as ps_s,
            tc.tile_pool(name="ps_v", bufs=3, space="PSUM") as ps_v,
            tc.tile_pool(name="ps_u", bufs=2, space="PSUM") as ps_u,
        ):
            # ---- constants ----
            nst_sb = constp.tile([128, 4, 2, 64], bf16)
            s2x_sb = constp.tile([128, 128], bf16)
            mask_sb = constp.tile([128, CB], bf16)
            id_sb = constp.tile([128, 128], bf16)
            id32_sb = constp.tile([128, 128], f32)
            id_r = constp.tile([128, 128], mybir.dt.float32r)
            itp_sb = constp.tile([D, H], bf16)
            xt_sb = xpool.tile([D, NBLK, BLK], bf16)
            nc.sync.dma_start(itp_sb[:], itp_d[:])
            nc.sync.dma_start(xt_sb[:, 0, :], xt_d[:, 0:BLK])
            nc.sync.dma_start(nst_sb[:].rearrange("p k g f -> p (k g f)"), nst_d[:])
            nc.sync.dma_start(s2x_sb[:], s2x_d[:])
            nc.sync.dma_start(mask_sb[:], mask_d[:])
            nc.sync.dma_start(id_sb[:], id_d[:])
            nc.sync.dma_start(id32_sb[:], id32_d[:])
            for blk in range(1, NBLK):
                nc.sync.dma_start(
                    xt_sb[:, blk, :], xt_d[:, blk * BLK:(blk + 1) * BLK]
                )

            # u'[p, c, t*BS+b] = 0.1 * u[b, t, 128c+p], staged per (blk, c)
            u_sb = upool.tile([128, C, TB], bf16)

            def u_matmul(blk, c):
                up = ps_u.tile([128, BLK], f32)
                nc.tensor.matmul(
                    up[:],
                    itp_sb[:, c * 128:(c + 1) * 128],
                    xt_sb[:, blk, :],
                    start=True, stop=True,
                )
                return up

            def u_copy(blk, c, up):
                dst = u_sb[:, c, blk * BLK:(blk + 1) * BLK]
                nc.vector.tensor_copy(dst, up[:])

            # one-time: f32r-rounded identity for the hx injection
            nc.vector.tensor_copy(id_r[:], id32_sb[:])

            # prologue: block 0 only (covers steps 0..STEPS_PER_BLK-1)
            for c in range(C):
                up = u_matmul(0, c)
                u_copy(0, c, up)

            # interleave schedule for blocks 1..NBLK-1:
            #   chunk c of block k rides step 64*(k-1) + 8 + 4*c
            u_sched = {}
            for k in range(1, NBLK):
                for c in range(C):
                    u_sched[(k - 1) * STEPS_PER_BLK + 8 + 4 * c] = (k, c)

            u_v = u_sb[:].rearrange("p c (t b) -> p c t b", b=BS)

            # ---- recurrence ----
            ring_tiles = []

            def ring_slot(t):
                return divmod(t, ring)

            def stage_out(t, vp_t):
                """ACT-copy h_t from PSUM into the ring; DMA full groups."""
                g, s = ring_slot(t)
                nc.scalar.activation(ring_tiles[g][:, s, :], vp_t[:], AF.Copy)
                if s == ring - 1 or t == T_steps - 1:
                    n_t = s + 1
                    nc.sync.dma_start(
                        out_d[:, g * ring * CB:(g * ring + n_t) * CB],
                        ring_tiles[g][:, :n_t, :].rearrange("p t f -> p (t f)"),
                    )

            vp_prev = None
            for t in range(T_steps):
                g, s = ring_slot(t)
                if s == 0:
                    rt = ringp.tile([128, ring, CB], f32, tag="ring")
                    ring_tiles.append(rt)

                usched = u_sched.get(t)
                if usched is not None:
                    up_pend = u_matmul(*usched)   # rides the tanh window
                    u_copy(*usched, up_pend)

                vp = ps_v.tile([128, CB], f32)
                if t == 0:
                    # h_0 = u'_0  (reference: h_1 = 0.1*u_0 with zero init)
                    nc.tensor.matmul(vp[:], id_sb[:], u_v[:, :, 0, :],
                                     start=True, stop=True)
                else:
                    # chain 1: th = tanh(h) directly off PSUM, cast to bf16
                    th = thp.tile([128, C, BS], bf16)
                    nc.scalar.activation(
                        th[:].rearrange("p c b -> p (c b)"), vp_prev[:], AF.Tanh
                    )
                    # off-chain: hx = 0.9*h + u'_t (DVE, f32r out, after tanh)
                    hx = hxp.tile([128, CB], mybir.dt.float32r)
                    nc.vector.scalar_tensor_tensor(
                        hx[:].rearrange("p (c b) -> p c b", b=BS),
                        vp_prev[:].rearrange("p (c b) -> p c b", b=BS),
                        DECAY, u_v[:, :, t, :], OP.mult, OP.add,
                    )
                    # off-chain: stage h_{t-1} to the output ring (ACT)
                    stage_out(t - 1, vp_prev)
                    # chain 2: srep2[64g+8j+r, :] = s_g replicated (8 bf16 MMs)
                    srep2 = ps_s.tile([128, CB], f32)
                    for k in range(4):
                        for gg in range(2):
                            nc.tensor.matmul(
                                srep2[64 * gg:64 * (gg + 1), :],
                                nst_sb[:, k, gg, :],
                                th[:, 2 * k + gg:2 * k + gg + 1, :]
                                .to_broadcast((128, C, BS)),
                                start=(k == 0), stop=(k == 3),
                            )
                    # inject hx = 0.9h + u' (single f32r identity matmul)
                    nc.tensor.matmul(vp[:], id_r[:], hx[:], start=True, stop=False)
                    # chain 3: blockdiag mask, psum -> sbuf, bf16
                    rhs2 = rhs2p.tile([128, CB], bf16)
                    nc.vector.tensor_tensor(rhs2[:], srep2[:], mask_sb[:], OP.mult)
                    # chain 4: vp += s2x^T @ rhs2
                    nc.tensor.matmul(vp[:], s2x_sb[:], rhs2[:], start=False, stop=True)

                vp_prev = vp

            stage_out(T_steps - 1, vp_prev)

    nc.compile()
    return nc


def prep_inputs(x, m, n, I, T_steps=T):
    """Host-side shard + layout prep (pure data marshaling)."""
    import ml_dtypes
    bf16 = ml_dtypes.bfloat16

    x = np.asarray(x, np.float32)
    m = np.asarray(m, np.float32)
    n = np.asarray(n, np.float32)
    I = np.asarray(I, np.float32)

    itp = np.ascontiguousarray((ALPHA * I).T).astype(bf16)      # [D, H]
    # nst[p, k, g, 8j+r] = n[256k+128g+p, r]  (replicated over j)
    nst = np.empty((128, 4, 2, 64), np.float32)
    for k in range(4):
        for g in range(2):
            blk = n[256 * k + 128 * g: 256 * k + 128 * g + 128, :]  # [128, 8]
            nst[:, k, g, :] = np.tile(blk, (1, 8))
    # s2x[64g+8j+r, p] = 0.1*m[128j+p, r]
    s2 = (ALPHA * m).reshape(C, 128, R).transpose(0, 2, 1).reshape(C * R, 128)
    s2x = np.tile(s2, (2, 1))                                   # [128, 128]
    maskx = np.tile(
        np.kron(np.eye(C, dtype=np.float32), np.ones((R, BS), np.float32)), (2, 1)
    )                                                           # [128, 64]

    nst_b = nst.reshape(128, 4 * 2 * 64).astype(bf16)
    s2x_b = s2x.astype(bf16)
    maskx_b = maskx.astype(bf16)
    eye = np.eye(128, dtype=np.float32)
    id_b = eye.astype(bf16)

    in_maps = []
    for core in range(NC):
        xs = x[core * BS:(core + 1) * BS, :T_steps]             # [BS, Ts, D]
        xt = np.ascontiguousarray(
            xs.transpose(2, 1, 0).reshape(D, T_steps * BS)
        ).astype(bf16)
        in_maps.append({
            "xt": xt, "itp": itp, "nst": nst_b, "s2x": s2x_b,
            "maskx": maskx_b, "id128": id_b, "id32": eye,
        })
    return in_maps


def unshard_out(res_core, T_steps=T):
    """[128, T*64] device layout -> [BS, T, H] full layout for one core."""
    a = res_core.reshape(128, T_steps, C, BS)        # [p, t, c, b]
    return np.ascontiguousarray(a.transpose(3, 1, 2, 0)).reshape(BS, T_steps, H)


def kernel(x, m, n, I):
    from concourse.bass_utils import run_bass_kernel_spmd

    if "nc" not in _CACHE:
        _CACHE["nc"] = build()
    nc = _CACHE["nc"]

    in_maps = prep_inputs(x, m, n, I)
    res = run_bass_kernel_spmd(nc, in_maps, core_ids=list(range(NC)))
    out = np.concatenate(
        [unshard_out(res.results[c]["out"]) for c in range(NC)], axis=0
    )
    return out


# revision 19
# speedup vs baseline: 1.0223x; 1.0186x over previous
"""Low-rank RNN (h' = 0.9h + 0.1*tanh(h) @ J^T + 0.1*u, J = m n^T rank-8)
on 8 Trainium2 NeuronCores, data-parallel over batch.

v6 layout per core (batch shard Bs=8, T=512, H=1024=8x128, D=128, R=8):
  state h lives in PSUM: vp[p, 8c+b] = h[b, 128c+p]
  per step chain:
    tanh (ACT, PSUM->SBUF, bf16)
    -> mm1: 8 bf16 matmuls in 4 rounds x 2 column-halves of the PE array;
       half g accumulates s_g = sum_k n_{2k+g}^T th_{2k+g} (j-replicated)
       into srep2[64g+8j+r, :]
    -> mask (DVE, PSUM->SBUF bf16): rhs2[64g+8j+r, 8c+b] = s_g[b,r]*[j==c]
    -> mm2: vp' += s2x^T @ rhs2 (bf16); s2x[64g+8j+r, p] = 0.1*m[128j+p, r]
  The linear part of the update rides PSUM accumulation as bf16 identity
  matmuls with NO per-step vector math:
    u'_t injected straight from the staged bf16 u tensor (exact);
    0.9*h injected from bf16 hi/lo ring copies of h (DVE: hi = bf16(h),
    lo = bf16(h - hi)) through two diagonal weights a = bf16(0.9),
    b = bf16(0.9 - a)  (a(hi+lo) + b*hi = 0.9h to ~2^-17).
  The hi/lo ring doubles as output staging: contiguous 4KB-per-partition
  DMAs of bf16 pairs; the host adds hi+lo and un-permutes.
  The u' staging matmuls are interleaved into the recurrence PE idle time,
  their PSUM->SBUF copies ride the ACT idle time.
"""

import numpy as np

B, T, D, H, R = 64, 512, 128, 1024, 8
NC = 8            # cores
BS = B // NC      # batch per core = 8
C = H // 128      # h chunks = 8
ALPHA = 0.1
DECAY = 1.0 - ALPHA

_CACHE = {}


def build(T_steps=T, ring=16, debug=False):
    import concourse.mybir as mybir
    import concourse.tile as tile
    from concourse import bacc

    f32 = mybir.dt.float32
    bf16 = mybir.dt.bfloat16
    AF = mybir.ActivationFunctionType
    OP = mybir.AluOpType

    nc = bacc.Bacc("TRN2", target_bir_lowering=False, debug=debug)

    TB = T_steps * BS                       # columns of xt / u
    BLK = min(512, TB)
    NBLK = TB // BLK
    STEPS_PER_BLK = BLK // BS

    xt_d = nc.dram_tensor("xt", [D, TB], bf16, kind="ExternalInput")
    itp_d = nc.dram_tensor("itp", [D, H], bf16, kind="ExternalInput")
    nst_d = nc.dram_tensor("nst", [128, 4 * 2 * 64], bf16, kind="ExternalInput")
    s2x_d = nc.dram_tensor("s2x", [128, 128], bf16, kind="ExternalInput")
    mask_d = nc.dram_tensor("maskx", [128, C * BS], bf16, kind="ExternalInput")
    id_d = nc.dram_tensor("id128", [128, 128], bf16, kind="ExternalInput")
    id32_d = nc.dram_tensor("id32", [128, 128], f32, kind="ExternalInput")
    out_d = nc.dram_tensor("out", [128, T_steps * C * BS], f32, kind="ExternalOutput")

    CB = C * BS  # 64

    with tile.TileContext(nc) as tc:
        with (
            tc.tile_pool(name="const", bufs=1) as constp,
            tc.tile_pool(name="upool", bufs=1) as upool,
            tc.tile_pool(name="xpool", bufs=1) as xpool,
            tc.tile_pool(name="th", bufs=3) as thp,
            tc.tile_pool(name="rhs2", bufs=3) as rhs2p,
            tc.tile_pool(name="hxp", bufs=3) as hxp,
            tc.tile_pool(name="ring", bufs=2) as ringp,
            tc.tile_pool(name="ps_s", bufs=2, space="PSUM") as ps_s,
            tc.tile_pool(name="ps_v", bufs=3, space="PSUM") as ps_v,
            tc.tile_pool(name="ps_u", bufs=2, space="PSUM") as ps_u,
        ):
            # ---- constants ----
            nst_sb = constp.tile([128, 4, 2, 64], bf16)
            s2x_sb = constp.tile([128, 128], bf16)
            mask_sb = constp.tile([128, CB], bf16)
            id_sb = constp.tile([128, 128], bf16)
            id32_sb = constp.tile([128, 128], f32)
            id_r = constp.tile([128, 128], mybir.dt.float32r)
            itp_sb = constp.tile([D, H], bf16)
            xt_sb = xpool.tile([D, NBLK, BLK], bf16)
            nc.sync.dma_start(itp_sb[:], itp_d[:])
            nc.sync.dma_start(xt_sb[:, 0, :], xt_d[:, 0:BLK])
            nc.sync.dma_start(nst_sb[:].rearrange("p k g f -> p (k g f)"), nst_d[:])
            nc.sync.dma_start(s2x_sb[:], s2x_d[:])
            nc.sync.dma_start(mask_sb[:], mask_d[:])
            nc.sync.dma_start(id_sb[:], id_d[:])
            nc.sync.dma_start(id32_sb[:], id32_d[:])
            for blk in range(1, NBLK):
                nc.sync.dma_start(
                    xt_sb[:, blk, :], xt_d[:, blk * BLK:(blk + 1) * BLK]
                )

            # u'[p, c, t*BS+b] = 0.1 * u[b, t, 128c+p], staged per (blk, c)
            u_sb = upool.tile([128, C, TB], bf16)

            def u_matmul(blk, c):
                up = ps_u.tile([128, BLK], f32)
                nc.tensor.matmul(
                    up[:],
                    itp_sb[:, c * 128:(c + 1) * 128],
                    xt_sb[:, blk, :],
                    start=True, stop=True,
                )
                return up

            def u_copy(blk, c, up):
                dst = u_sb[:, c, blk * BLK:(blk + 1) * BLK]
                if c % 2 == 0:
                    nc.scalar.activation(dst, up[:], AF.Copy)
                else:
                    nc.vector.tensor_copy(dst, up[:])

            # one-time: f32r-rounded identity for the hx injection
            nc.vector.tensor_copy(id_r[:], id32_sb[:])

            # prologue: block 0 only (covers steps 0..STEPS_PER_BLK-1)
            for c in range(C):
                up = u_matmul(0, c)
                u_copy(0, c, up)

            # interleave schedule for blocks 1..NBLK-1:
            #   chunk c of block k rides step 64*(k-1) + 8 + 4*c
            u_sched = {}
            for k in range(1, NBLK):
                for c in range(C):
                    u_sched[(k - 1) * STEPS_PER_BLK + 8 + 4 * c] = (k, c)

            u_v = u_sb[:].rearrange("p c (t b) -> p c t b", b=BS)

            # ---- recurrence ----
            ring_tiles = []

            def ring_slot(t):
                return divmod(t, ring)

            def stage_out(t, vp_t):
                """ACT-copy h_t from PSUM into the ring; DMA full groups."""
                g, s = ring_slot(t)
                nc.scalar.activation(ring_tiles[g][:, s, :], vp_t[:], AF.Copy)
                if s == ring - 1 or t == T_steps - 1:
                    n_t = s + 1
                    nc.sync.dma_start(
                        out_d[:, g * ring * CB:(g * ring + n_t) * CB],
                        ring_tiles[g][:, :n_t, :].rearrange("p t f -> p (t f)"),
                    )

            vp_prev = None
            for t in range(T_steps):
                g, s = ring_slot(t)
                if s == 0:
                    rt = ringp.tile([128, ring, CB], f32, tag="ring")
                    ring_tiles.append(rt)

                usched = u_sched.get(t)
                if usched is not None:
                    up_pend = u_matmul(*usched)   # rides the tanh window
                    u_copy(*usched, up_pend)

                vp = ps_v.tile([128, CB], f32)
                if t == 0:
                    # h_0 = u'_0  (reference: h_1 = 0.1*u_0 with zero init)
                    nc.tensor.matmul(vp[:], id_sb[:], u_v[:, :, 0, :],
                                     start=True, stop=True)
                else:
                    # chain 1: th = tanh(h) directly off PSUM, cast to bf16
                    th = thp.tile([128, C, BS], bf16)
                    nc.scalar.activation(
                        th[:].rearrange("p c b -> p (c b)"), vp_prev[:], AF.Tanh
                    )
                    # off-chain: hx = 0.9*h + u'_t (DVE, f32r out, after tanh)
                    hx = hxp.tile([128, CB], mybir.dt.float32r)
                    nc.vector.scalar_tensor_tensor(
                        hx[:].rearrange("p (c b) -> p c b", b=BS),
                        vp_prev[:].rearrange("p (c b) -> p c b", b=BS),
                        DECAY, u_v[:, :, t, :], OP.mult, OP.add,
                    )
                    # off-chain: stage h_{t-1} to the output ring (ACT)
                    stage_out(t - 1, vp_prev)
                    # chain 2: srep2[64g+8j+r, :] = s_g replicated (8 bf16 MMs)
                    srep2 = ps_s.tile([128, CB], f32)
                    for k in range(4):
                        for gg in range(2):
                            nc.tensor.matmul(
                                srep2[64 * gg:64 * (gg + 1), :],
                                nst_sb[:, k, gg, :],
                                th[:, 2 * k + gg:2 * k + gg + 1, :]
                                .to_broadcast((128, C, BS)),
                                start=(k == 0), stop=(k == 3),
                            )
                    # inject hx = 0.9h + u' (single f32r identity matmul)
                    nc.tensor.matmul(vp[:], id_r[:], hx[:], start=True, stop=False)
                    # chain 3: blockdiag mask, psum -> sbuf, bf16
                    rhs2 = rhs2p.tile([128, CB], bf16)
                    nc.vector.tensor_tensor(rhs2[:], srep2[:], mask_sb[:], OP.mult)
                    # chain 4: vp += s2x^T @ rhs2
                    nc.tensor.matmul(vp[:], s2x_sb[:], rhs2[:], start=False, stop=True)

                vp_prev = vp

            stage_out(T_steps - 1, vp_prev)

    nc.compile()
    return nc


def prep_inputs(x, m, n, I, T_steps=T):
    """Host-side shard + layout prep (pure data marshaling)."""
    import ml_dtypes
    bf16 = ml_dtypes.bfloat16

    x = np.asarray(x, np.float32)
    m = np.asarray(m, np.float32)
    n = np.asarray(n, np.float32)
    I = np.asarray(I, np.float32)

    itp = np.ascontiguousarray((ALPHA * I).T).astype(bf16)      # [D, H]
    # nst[p, k, g, 8j+r] = n[256k+128g+p, r]  (replicated over j)
    nst = np.empty((128, 4, 2, 64), np.float32)
    for k in range(4):
        for g in range(2):
            blk = n[256 * k + 128 * g: 256 * k + 128 * g + 128, :]  # [128, 8]
            nst[:, k, g, :] = np.tile(blk, (1, 8))
    # s2x[64g+8j+r, p] = 0.1*m[128j+p, r]
    s2 = (ALPHA * m).reshape(C, 128, R).transpose(0, 2, 1).reshape(C * R, 128)
    s2x = np.tile(s2, (2, 1))                                   # [128, 128]
    maskx = np.tile(
        np.kron(np.eye(C, dtype=np.float32), np.ones((R, BS), np.float32)), (2, 1)
    )                                                           # [128, 64]

    nst_b = nst.reshape(128, 4 * 2 * 64).astype(bf16)
    s2x_b = s2x.astype(bf16)
    maskx_b = maskx.astype(bf16)
    eye = np.eye(128, dtype=np.float32)
    id_b = eye.astype(bf16)

    in_maps = []
    for core in range(NC):
        xs = x[core * BS:(core + 1) * BS, :T_steps]             # [BS, Ts, D]
        xt = np.ascontiguousarray(
            xs.transpose(2, 1, 0).reshape(D, T_steps * BS)
        ).astype(bf16)
        in_maps.append({
            "xt": xt, "itp": itp, "nst": nst_b, "s2x": s2x_b,
            "maskx": maskx_b, "id128": id_b, "id32": eye,
        })
    return in_maps


def unshard_out(res_core, T_steps=T):
    """[128, T*64] device layout -> [BS, T, H] full layout for one core."""
    a = res_core.reshape(128, T_steps, C, BS)        # [p, t, c, b]
    return np.ascontiguousarray(a.transpose(3, 1, 2, 0)).reshape(BS, T_steps, H)


def kernel(x, m, n, I):
    from concourse.bass_utils import run_bass_kernel_spmd

    if "nc" not in _CACHE:
        _CACHE["nc"] = build()
    nc = _CACHE["nc"]

    in_maps = prep_inputs(x, m, n, I)
    res = run_bass_kernel_spmd(nc, in_maps, core_ids=list(range(NC)))
    out = np.concatenate(
        [unshard_out(res.results[c]["out"]) for c in range(NC)], axis=0
    )
    return out
